# revision 40
# baseline (speedup 1.0000x reference)
"""Trainium2 Bass kernel for the CustomS5Block problem.

Strategy
--------
Data-parallel: batch 8 -> one batch element per NeuronCore.

Math: l1 has input dim 1, so u[l,h] = x[l]*l1w[h] (+l1_b) and
Bu[l,p] = x[l]*bb[p] with bb = B_bar @ l1w.  The diagonal S5 scan with
constant coefficient lam_bar = r*e^{i*phi} reduces to exponential
filters of the scalar signal x:

    K[l,p] = sum_{j<=l} r^{l-j} e^{i(l-j)phi} x[j]
    xs     = bb * K                       (folded into C on the host)

Rotation decomposition (exact, numerically stable):
    Sc[l,p] = r*Sc[l-1,p] + cos(l*phi_p)*x[l]     (tensor_tensor_scan)
    Ss[l,p] = r*Ss[l-1,p] + sin(l*phi_p)*x[l]
    Kr = ct*Sc + st*Ss ;  Ki = st*Sc - ct*Ss      (DVE elementwise)

Key algebraic fold: h2 = dec@f + dec_b + h1 is consumed ONLY by the
l2 head, so  out = l2@h2 + l2_b collapses to

    out[l] = v.f[:,l] + l2.th[:,l] + (l2.l1w)*x[l] + c0
    v  = l2 @ dec            (host, [CE])
    c0 = l2.dec_b + l2_b + l2.l1_b

which deletes the entire [CE->H] dec GEMM (2.7 GMAC/core) and h2.

Engines: PE does C-proj (bf16) + enc (f32r) + v-dot/l2th (f32r);
DVE does d1-muls (bf16 2x), scans (fp32), recombination; GpSimd does
so (=y+D*l1w*x), h1 (=th+l1w*x), orow (=psum+l2.l1w*x); ACT does the
tanh's (eps merged [128, 2*lc] across 2 PSUM banks).

Software pipeline: part_a two chunks ahead (DVE), C-projection one
chunk ahead (PE), v-dot tail + output deferred past the next
C-projection.  First chunks are smaller (256) to shorten the fill.
"""
import numpy as np
import ml_dtypes

import concourse.bass as bass
import concourse.tile as tile
from concourse import mybir
from concourse.bass_utils import run_bass_kernel_spmd

dt = mybir.dt
AF = mybir.ActivationFunctionType
OP = mybir.AluOpType

L = 4096
H = 256             # model width (2 tiles of 128)
P = 256             # state dim (2 tiles of 128)
CE = 2560           # 10*H
NCC = CE // 128     # 20
NG = NCC // 2       # 10 cc-pairs per chunk
NF8 = 10            # cc blocks (smallest |v|, permuted last) in fp8
NBF = NCC - NF8     # bf16 cc blocks

# chunk schedule: smaller leading chunks shorten the DVE fill
CHUNKS = [256, 256, 512, 512, 512, 512, 512, 512, 512]
assert sum(CHUNKS) == L
NCH = len(CHUNKS)
LCMAX = max(CHUNKS)

_ws_ctr = [0]


def _split_multi_waits(nc, max_waits=1):
    """walrus encodes at most one sync wait per compute instruction;
    hoist extras onto single-wait EventSemaphore ops on the same engine."""
    for func in nc.m.functions:
        for blk in func.blocks:
            new_insts = []
            for inst in blk.instructions:
                si = inst.sync_info
                if si is not None and len(si.on_wait) > max_waits:
                    waits = list(si.on_wait)
                    extra, keep = waits[:-max_waits], waits[-max_waits:]
                    for w in extra:
                        _ws_ctr[0] += 1
                        ev = mybir.InstEventSemaphore(
                            name=f"WSPLIT-{_ws_ctr[0]}", ins=[], outs=[],
                            engine=inst.engine)
                        ev.sync_info = mybir.SyncInfo(on_wait=[w], on_update=[])
                        new_insts.append(ev)
                    inst.sync_info = mybir.SyncInfo(
                        on_wait=keep, on_update=list(si.on_update))
                new_insts.append(inst)
            blk.instructions = new_insts
    return nc


def derive_host_tables(l1_w, l1_b, lam_re, lam_im, B_re, B_im, C_re, C_im,
                       D, log_step):
    """Parameter-only precompute (no dependence on x)."""
    l1w = np.asarray(l1_w, np.float32)[:, 0]
    l1b = np.asarray(l1_b, np.float32)
    lam = (np.asarray(lam_re, np.float32)
           + 1j * np.asarray(lam_im, np.float32)).astype(np.complex64)
    step = np.exp(np.asarray(log_step, np.float32)).astype(np.complex64)
    lam_bar = np.exp(lam * step)                       # complex64 [P]
    Bm = (np.asarray(B_re, np.float32)
          + 1j * np.asarray(B_im, np.float32)).astype(np.complex64)
    B_bar = ((lam_bar - 1.0) / lam)[:, None] * Bm      # [P, H]
    bb = B_bar @ l1w.astype(np.complex64)              # [P]

    r = np.abs(lam_bar).astype(np.float64)
    phi = np.angle(lam_bar).astype(np.float64)
    ls = np.arange(L, dtype=np.float64)
    ang = ls[None, :] * phi[:, None]                   # [P, L]
    ct = np.cos(ang).astype(np.float32)
    st = np.sin(ang).astype(np.float32)
    r32 = r.astype(np.float32)

    Cm = (np.asarray(C_re, np.float32)
          + 1j * np.asarray(C_im, np.float32)).astype(np.complex64)
    Ct = Cm * bb[None, :]                              # [H, P]
    Wr = (2.0 * Ct.real).T.astype(np.float32).copy()   # [P, H]
    Wi = (-2.0 * Ct.imag).T.astype(np.float32).copy()  # [P, H]
    dl1 = (np.asarray(D, np.float32) * l1w).astype(np.float32)

    # scan-side correction for nonzero l1_b: Bu gains the constant
    # bbb[p] = B_bar @ l1_b, whose scan is a closed-form geometric sum.
    if np.any(l1b != 0):
        bbb = (B_bar @ l1b.astype(np.complex64)).astype(np.complex128)
        lb = lam_bar.astype(np.complex128)
        pw = np.empty((P, L), np.complex128)
        acc = np.ones(P, np.complex128)
        for j in range(L):
            pw[:, j] = acc
            acc = acc * lb
        g = np.cumsum(pw, axis=1)
        xs_c = bbb[:, None] * g
        yc = 2.0 * np.real(Cm.astype(np.complex128) @ xs_c)  # [H, L]
        yc = yc.astype(np.float32)
    else:
        yc = None

    # host-side packing (bf16 tables):
    # tabs[row, chunk-offsets]: per chunk k and kp: [ct | st] of [128, lc]
    bf = ml_dtypes.bfloat16
    tabs = np.empty((128, 4 * L), bf)
    offs = []
    off = 0
    lo = 0
    for lc in CHUNKS:
        offs.append(off)
        for kp in range(2):
            tabs[:, off:off + lc] = ct[kp * 128:(kp + 1) * 128,
                                       lo:lo + lc].astype(bf)
            off += lc
            tabs[:, off:off + lc] = st[kp * 128:(kp + 1) * 128,
                                       lo:lo + lc].astype(bf)
            off += lc
        lo += lc
    rbc = np.repeat(r32[:, None], LCMAX, axis=1).copy()   # [P, LCMAX]
    # wc blocks: (term t in {r,i}, kp, hh) -> [128,128]
    wc = np.empty((128, 1024), bf)
    for t, W in enumerate((Wr, Wi)):
        for kp in range(2):
            for hh in range(2):
                blkidx = (t * 2 + kp) * 2 + hh
                wc[:, blkidx * 128:(blkidx + 1) * 128] = \
                    W[kp * 128:(kp + 1) * 128,
                      hh * 128:(hh + 1) * 128].astype(bf)
    return dict(tabs=tabs, tab_offs=offs, rbc=rbc, wc=wc,
                dl1row=dl1[None, :].astype(bf).copy(), l1w=l1w,
                l1b=l1b, yc=yc)


def build_program(flags):
    has_yc = flags['has_yc']
    has_encb = flags['has_encb']
    has_c0 = flags['has_c0']
    has_l1b = flags['has_l1b']
    tab_offs = flags['tab_offs']

    nc = bass.Bass("TRN2", target_bir_lowering=False, debug=False,
                   num_devices=8)
    f32, f32r, bf16 = dt.float32, dt.float32r, dt.bfloat16
    f8 = dt.float8e4
    PM = mybir.MatmulPerfMode

    xbs = nc.dram_tensor("xbs", [1, L], bf16, kind="ExternalInput")
    tabs = nc.dram_tensor("tabs", [128, 4 * L], bf16, kind="ExternalInput")
    rbc = nc.dram_tensor("rbc", [P, LCMAX], f32, kind="ExternalInput")
    wc = nc.dram_tensor("wc", [128, 1024], bf16, kind="ExternalInput")
    dl1row = nc.dram_tensor("dl1row", [1, H], bf16, kind="ExternalInput")
    encbf = nc.dram_tensor("encbf", [128, NBF * 256], bf16,
                           kind="ExternalInput")
    encf8 = nc.dram_tensor("encf8", [128, NF8 * 256], f8,
                           kind="ExternalInput")
    encb = nc.dram_tensor("encb", [128, NCC], f32, kind="ExternalInput") \
        if has_encb else None
    vpk = nc.dram_tensor("vpk", [128, NCC], f32r, kind="ExternalInput")
    l2v = nc.dram_tensor("l2v", [128, 2], f32r, kind="ExternalInput")
    l1wc = nc.dram_tensor("l1wc", [128, 2], f32, kind="ExternalInput")
    l1bc = nc.dram_tensor("l1bc", [128, 2], f32, kind="ExternalInput") \
        if has_l1b else None
    kap = nc.dram_tensor("kap", [1, 2], f32, kind="ExternalInput")
    kapb = nc.dram_tensor("kapb", [1, 1], bf16, kind="ExternalInput")
    ycd = nc.dram_tensor("yc", [H, L], f32, kind="ExternalInput") \
        if has_yc else None
    out = nc.dram_tensor("out", [1, L], f32, kind="ExternalOutput")

    los = np.cumsum([0] + CHUNKS).tolist()

    with tile.TileContext(nc) as tc:
        with tc.tile_pool(name="const", bufs=1) as cpool, \
             tc.tile_pool(name="stream", bufs=3) as spool, \
             tc.tile_pool(name="xpool", bufs=4) as xpool, \
             tc.tile_pool(name="work", bufs=2) as wpool, \
             tc.tile_pool(name="fpool", bufs=4) as fpool, \
             tc.tile_pool(name="ps_y", bufs=2, space="PSUM") as ps_y, \
             tc.tile_pool(name="ps_e", bufs=2, space="PSUM") as ps_e, \
             tc.tile_pool(name="ps_o", bufs=2, space="PSUM") as ps_o:

            # ---------- small constants (first: unblock the scan) ----------
            wc_sb = cpool.tile([128, 1024], bf16)
            nc.gpsimd.dma_start(out=wc_sb[:], in_=wc[:])
            rbc_sb = cpool.tile([128, 2 * LCMAX], f32)
            nc.gpsimd.dma_start(out=rbc_sb[:, 0:LCMAX], in_=rbc[0:128, :])
            nc.gpsimd.dma_start(out=rbc_sb[:, LCMAX:2 * LCMAX],
                                in_=rbc[128:256, :])
            dl1_sb = cpool.tile([1, H], bf16)
            nc.gpsimd.dma_start(out=dl1_sb[:], in_=dl1row[:])
            l1w_sb = cpool.tile([128, 2], f32)
            nc.gpsimd.dma_start(out=l1w_sb[:], in_=l1wc[:])
            v_sb = cpool.tile([128, NCC], f32r)
            nc.gpsimd.dma_start(out=v_sb[:], in_=vpk[:])
            l2v_sb = cpool.tile([128, 2], f32r)
            nc.gpsimd.dma_start(out=l2v_sb[:], in_=l2v[:])
            kap_sb = cpool.tile([1, 2], f32)
            nc.gpsimd.dma_start(out=kap_sb[:], in_=kap[:])
            kapb_sb = cpool.tile([1, 1], bf16)
            nc.gpsimd.dma_start(out=kapb_sb[:], in_=kapb[:])
            if has_encb:
                encb_sb = cpool.tile([128, NCC], f32)
                nc.gpsimd.dma_start(out=encb_sb[:], in_=encb[:])
            if has_l1b:
                l1b_sb = cpool.tile([128, 2], f32)
                nc.gpsimd.dma_start(out=l1b_sb[:], in_=l1bc[:])

            # PE warm-up tiles: memset, no DMA dependency
            wa = cpool.tile([128, 128], f32r)
            nc.vector.memset(wa[:].bitcast(f32), 0.0)
            wb = cpool.tile([128, 512], f32r)
            nc.vector.memset(wb[:].bitcast(f32), 0.0)

            def stream_chunk(i):
                lo, lc = los[i], CHUNKS[i]
                xb = xpool.tile([128, 2 * LCMAX], bf16, tag="xb",
                                name=f"xb_{i}")
                nc.sync.dma_start(
                    out=xb[:, 0:lc],
                    in_=xbs[0:1, lo:lo + lc].broadcast_to([128, lc]))
                nc.sync.dma_start(
                    out=xb[:, lc:2 * lc],
                    in_=xbs[0:1, lo:lo + lc].broadcast_to([128, lc]))
                tab_t = []
                for kp in range(2):
                    t = spool.tile([128, 2 * LCMAX], bf16, tag=f"tab{kp}",
                                   name=f"tab{kp}_{i}")
                    off = tab_offs[i] + kp * 2 * CHUNKS[i]
                    nc.sync.dma_start(out=t[:, 0:2 * lc],
                                      in_=tabs[:, off:off + 2 * lc])
                    tab_t.append(t)
                yc_t = None
                if has_yc:
                    yc_t = spool.tile([128, 2 * LCMAX], f32, tag="yc",
                                      name=f"yc_{i}")
                    nc.sync.dma_start(
                        out=yc_t[:, 0:2 * lc].rearrange(
                            "p (hh l) -> p hh l", hh=2),
                        in_=ycd.ap().rearrange("(hh p) l -> p hh l", p=128)
                        [:, :, lo:lo + lc])
                return tab_t, xb, yc_t

            # warm-up helper: matmul junk to keep the PE HAM un-throttled
            warm_ctr = [0]

            def warm(n, moving=None, mlen=512):
                for _ in range(n):
                    warm_ctr[0] += 1
                    wps = ps_e.tile([128, 2 * LCMAX], f32, tag="e",
                                    name=f"warm{warm_ctr[0]}")
                    mv = wb[:, 0:mlen] if moving is None else moving
                    nc.tensor.matmul(wps[:, 0:mlen], wa[:], mv,
                                     start=True, stop=True)

            prev_sc = [None, None]
            prev_ss = [None, None]
            prev_lc = [0]

            def part_a_d1(i, streams):
                """d1 muls for chunk i (DVE): [d1c|d1s] in one op per
                kp against the twice-replicated x tile."""
                lc = CHUNKS[i]
                tab_t, xb, yc_t = streams
                d1 = []
                for kp in range(2):
                    d1cs = wpool.tile([128, 2 * LCMAX], bf16,
                                      tag=f"d1cs{kp}", name=f"d1cs{kp}_{i}")
                    nc.vector.tensor_mul(d1cs[:, 0:2 * lc],
                                         tab_t[kp][:, 0:2 * lc],
                                         xb[:, 0:2 * lc])
                    d1.append(d1cs)
                return d1

            def part_a(i, streams, d1=None):
                """scans + recombination for chunk i (DVE only)."""
                lc = CHUNKS[i]
                tab_t, xb, yc_t = streams
                if d1 is None:
                    d1 = part_a_d1(i, streams)
                scss_t = []
                for kp in range(2):
                    d1cs = d1[kp]
                    r_ap = rbc_sb[:, kp * LCMAX:kp * LCMAX + lc]
                    scss = wpool.tile([128, 2 * LCMAX], f32, tag=f"scss{kp}",
                                      name=f"scss{kp}_{i}")
                    init_c = 0.0 if i == 0 else \
                        prev_sc[kp][:, prev_lc[0] - 1:prev_lc[0]]
                    nc.vector.tensor_tensor_scan(
                        scss[:, 0:lc], r_ap, d1cs[:, 0:lc], init_c,
                        OP.mult, OP.add)
                    init_s = 0.0 if i == 0 else \
                        prev_ss[kp][:, 2 * prev_lc[0] - 1:2 * prev_lc[0]]
                    nc.vector.tensor_tensor_scan(
                        scss[:, lc:2 * lc], r_ap, d1cs[:, lc:2 * lc], init_s,
                        OP.mult, OP.add)
                    scss_t.append(scss)
                prev_sc[:] = scss_t
                prev_ss[:] = scss_t
                prev_lc[0] = lc

                kr_t, ki_t = [], []
                for kp in range(2):
                    ct_ap = tab_t[kp][:, 0:lc]
                    st_ap = tab_t[kp][:, lc:2 * lc]
                    sc_ap = scss_t[kp][:, 0:lc]
                    ss_ap = scss_t[kp][:, lc:2 * lc]
                    # m1 = [ct*Sc | st*Ss] in one paired op, then
                    # Kr = ct*Sc + st*Ss ; Ki = st*Sc - ct*Ss
                    m1 = wpool.tile([128, 2 * LCMAX], bf16, tag=f"m1{kp}",
                                    name=f"m1{kp}_{i}")
                    nc.vector.tensor_mul(m1[:, 0:2 * lc],
                                         tab_t[kp][:, 0:2 * lc],
                                         scss_t[kp][:, 0:2 * lc])
                    kr = wpool.tile([128, LCMAX], bf16, tag=f"kr{kp}",
                                    name=f"kr{kp}_{i}")
                    nc.vector.tensor_add(kr[:, 0:lc], m1[:, 0:lc],
                                         m1[:, lc:2 * lc])
                    s2 = wpool.tile([128, LCMAX], bf16, tag=f"s2{kp}",
                                    name=f"s2{kp}_{i}")
                    ki = wpool.tile([128, LCMAX], bf16, tag=f"ki{kp}",
                                    name=f"ki{kp}_{i}")
                    nc.vector.tensor_mul(s2[:, 0:lc], st_ap, sc_ap)
                    nc.vector.tensor_mul(ki[:, 0:lc], ct_ap, ss_ap)
                    nc.vector.tensor_sub(ki[:, 0:lc], s2[:, 0:lc], ki[:, 0:lc])
                    kr_t.append(kr)
                    ki_t.append(ki)
                return kr_t, ki_t, xb, yc_t

            def cproj(i, pa):
                """C-projection (PE, with the D*l1w*x rank-1 term folded
                in as a K=1 matmul so th reads PSUM directly) + th (ACT)
                + h1 (DVE stt, bf16 + fp8 shadow at equal cost)."""
                lc = CHUNKS[i]
                kr_t, ki_t, xb, yc_t = pa
                th_t, h1_t = [], []
                h1f = wpool.tile([128, 2 * LCMAX], f8, tag="h1f",
                                 name=f"h1f_{i}") if NF8 > 0 else None
                for hh in range(2):
                    yps = ps_y.tile([128, LCMAX], f32, tag="y",
                                    name=f"y{hh}_{i}")
                    for mi, ks in enumerate((kr_t, ki_t)):
                        for kp in range(2):
                            blkidx = (mi * 2 + kp) * 2 + hh
                            nc.tensor.matmul(
                                yps[:, 0:lc],
                                wc_sb[:, blkidx * 128:(blkidx + 1) * 128],
                                ks[kp][:, 0:lc], start=(mi == 0 and kp == 0),
                                stop=False)
                    # D*l1w*x rank-1 term as a K=1 matmul
                    nc.tensor.matmul(
                        yps[:, 0:lc], dl1_sb[0:1, hh * 128:(hh + 1) * 128],
                        xb[0:1, 0:lc], start=False, stop=True)
                    if has_yc:
                        nc.vector.tensor_add(
                            yps[:, 0:lc], yps[:, 0:lc],
                            yc_t[:, hh * lc:(hh + 1) * lc])
                    th = wpool.tile([128, LCMAX], f32r, tag=f"th{hh}",
                                    name=f"th{hh}_{i}")
                    nc.scalar.activation(th[:, 0:lc],
                                         yps[:, 0:lc], AF.Tanh)
                    h1 = wpool.tile([128, LCMAX], bf16, tag=f"h1{hh}",
                                    name=f"h1{hh}_{i}")
                    nc.vector.scalar_tensor_tensor(
                        h1[:, 0:lc], xb[:, 0:lc],
                        l1w_sb[:, hh:hh + 1], th[:, 0:lc].bitcast(f32),
                        OP.mult, OP.add)
                    if has_l1b:
                        nc.vector.tensor_scalar_add(
                            h1[:, 0:lc], h1[:, 0:lc],
                            l1b_sb[:, hh:hh + 1])
                    if h1f is not None:
                        if has_l1b:
                            nc.vector.tensor_copy(
                                h1f[:, hh * lc:hh * lc + lc], h1[:, 0:lc])
                        else:
                            nc.vector.scalar_tensor_tensor(
                                h1f[:, hh * lc:hh * lc + lc], xb[:, 0:lc],
                                l1w_sb[:, hh:hh + 1],
                                th[:, 0:lc].bitcast(f32),
                                OP.mult, OP.add)
                    th_t.append(th)
                    h1_t.append(h1)
                return th_t, h1_t, h1f, xb

            # ------------- prologue -------------
            streams = [None] * NCH
            pa = [None] * NCH
            streams[0] = stream_chunk(0)
            streams[1] = stream_chunk(1)
            warm(12)

            # large resident weights
            encbf_sb = cpool.tile([128, NBF * 256], bf16)
            nc.gpsimd.dma_start(out=encbf_sb[:], in_=encbf[:])
            encf8_sb = cpool.tile([128, NF8 * 256], f8)
            nc.gpsimd.dma_start(out=encf8_sb[:], in_=encf8[:])

            warm(6)
            pa[0] = part_a(0, streams[0])
            warm(10)
            pa[1] = part_a(1, streams[1])
            warm(10)

            cp = [None] * NCH
            cp[0] = cproj(0, pa[0])

            pending_tail = None

            def mlp(i, orow, mid=None):
                lo, lc = los[i], CHUNKS[i]
                th_t, h1_t, h1f, xb = cp[i]
                for hh in range(2):
                    nc.tensor.matmul(orow[0:1, 0:lc],
                                     l2v_sb[:, hh:hh + 1],
                                     th_t[hh][:, 0:lc],
                                     start=(hh == 0), stop=False)
                # (l2.l1w)*x rank-1 term as a K=1 matmul
                nc.tensor.matmul(orow[0:1, 0:lc], kapb_sb[0:1, 0:1],
                                 xb[0:1, 0:lc], start=False, stop=False)
                h1f2 = None if h1f is None else \
                    h1f[:, 0:2 * lc].rearrange("p (two l) -> p two l", two=2)
                f_prev = None
                for g in range(NG):
                    eps = ps_e.tile([128, 2 * LCMAX], f32, tag="e",
                                    name=f"eps{g}_{i}")
                    for half in range(2):
                        cc = 2 * g + half
                        if cc >= NBF:
                            j = cc - NBF
                            nc.tensor.matmul(
                                eps[:, half * lc:half * lc + lc],
                                encf8_sb[:, j * 256:(j + 1) * 256].rearrange(
                                    "p (two m) -> p two m", two=2),
                                h1f2, start=True, stop=True,
                                perf_mode=PM.DoubleRow)
                            continue
                        for kh in range(2):
                            nc.tensor.matmul(
                                eps[:, half * lc:half * lc + lc],
                                encbf_sb[:, cc * 256 + kh * 128:
                                         cc * 256 + (kh + 1) * 128],
                                h1_t[kh][:, 0:lc],
                                start=(kh == 0), stop=(kh == 1))
                    f_t = fpool.tile([128, 2 * LCMAX], f32r, tag="f",
                                     name=f"f{g}_{i}")
                    if has_encb:
                        for half in range(2):
                            cc = 2 * g + half
                            nc.scalar.activation(
                                f_t[:, half * lc:half * lc + lc],
                                eps[:, half * lc:half * lc + lc], AF.Tanh,
                                bias=encb_sb[:, cc:cc + 1])
                    else:
                        nc.scalar.activation(f_t[:, 0:2 * lc],
                                             eps[:, 0:2 * lc], AF.Tanh)
                    if f_prev is not None:
                        gp = g - 1
                        for half in range(2):
                            cc = 2 * gp + half
                            nc.tensor.matmul(
                                orow[0:1, 0:lc], v_sb[:, cc:cc + 1],
                                f_prev[:, half * lc:half * lc + lc],
                                start=False, stop=False)
                    f_prev = f_t
                    if g == 2 and mid is not None:
                        mid()
                return f_prev

            for i in range(NCH):
                lo, lc = los[i], CHUNKS[i]
                xb = cp[i][3]

                # streams two chunks ahead; d1 muls go on the DVE queue
                # before h1 so the scan pipeline isn't head-of-line blocked
                d1_next = None
                if i + 2 < NCH:
                    streams[i + 2] = stream_chunk(i + 2)
                    d1_next = part_a_d1(i + 2, streams[i + 2])

                orow = ps_o.tile([1, LCMAX], f32, tag="or", name=f"or_{i}")

                # During the fill the DVE scan pipeline lags the PE, so
                # the next C-projection would head-of-line-block the PE;
                # emit it after this chunk's MLP instead.  From chunk 3
                # on (DVE caught up) emit it first (steady state).  The
                # deferred v-dot tail goes after the next C-projection so
                # the PE never waits on the last tanh group.
                early = i < 3
                if early:
                    if pending_tail is not None:
                        pending_tail()
                        pending_tail = None
                    f_prev = mlp(i, orow)
                    if i + 1 < NCH:
                        cp[i + 1] = cproj(i + 1, pa[i + 1])
                else:
                    if pending_tail is not None:
                        pending_tail()
                        pending_tail = None

                    def mid(i=i):
                        if i + 1 < NCH:
                            cp[i + 1] = cproj(i + 1, pa[i + 1])
                    f_prev = mlp(i, orow, mid=mid)

                if i + 2 < NCH:
                    pa[i + 2] = part_a(i + 2, streams[i + 2], d1=d1_next)

                def tail(i=i, lo=lo, lc=lc, orow=orow, f_prev=f_prev):
                    for half in range(2):
                        cc = 2 * (NG - 1) + half
                        nc.tensor.matmul(
                            orow[0:1, 0:lc], v_sb[:, cc:cc + 1],
                            f_prev[:, half * lc:half * lc + lc],
                            start=False, stop=(half == 1))
                    og = wpool.tile([1, LCMAX], f32, tag="og",
                                    name=f"og_{i}")
                    if has_c0:
                        nc.scalar.activation(og[0:1, 0:lc], orow[0:1, 0:lc],
                                             AF.Identity,
                                             bias=kap_sb[0:1, 1:2])
                    else:
                        nc.scalar.activation(og[0:1, 0:lc], orow[0:1, 0:lc],
                                             AF.Identity)
                    nc.sync.dma_start(out=out[0:1, lo:lo + lc],
                                      in_=og[0:1, 0:lc])
                pending_tail = tail

            pending_tail()

    _split_multi_waits(nc)
    return nc


def kernel(x, l1_w, l1_b, lam_re, lam_im, B_re, B_im, C_re, C_im, D,
           log_step, ff_enc_w, ff_enc_b, ff_dec_w, ff_dec_b, l2_w, l2_b):
    bf = ml_dtypes.bfloat16
    e4 = ml_dtypes.float8_e4m3fn
    x = np.asarray(x, np.float32)
    B = x.shape[0]
    t = derive_host_tables(l1_w, l1_b, lam_re, lam_im, B_re, B_im,
                           C_re, C_im, D, log_step)

    l2row = np.asarray(l2_w, np.float64)[0]            # [H]
    dec_w = np.asarray(ff_dec_w, np.float64)           # [H, CE]
    v = (l2row @ dec_w).astype(np.float32)             # [CE]

    # permute MLP channels by |v|: the NF8*128 smallest go to the fp8
    # cc blocks (placed last), so fp8 quantization noise lands on the
    # channels with the least output weight.
    perm = np.argsort(np.abs(v))                       # ascending |v|
    perm = np.concatenate([perm[NF8 * 128:], perm[:NF8 * 128]])
    v_p = v[perm]
    vpk = v_p.reshape(NCC, 128).T.copy()               # [128, NCC]

    enc_w = np.asarray(ff_enc_w, np.float32)[perm]     # [CE, H] permuted
    ET = enc_w.T                                       # [H, CE]
    encbf = np.empty((128, NBF * 256), np.float32)
    for cc in range(NBF):
        for kh in range(2):
            encbf[:, cc * 256 + kh * 128:cc * 256 + (kh + 1) * 128] = \
                ET[kh * 128:(kh + 1) * 128, cc * 128:(cc + 1) * 128]
    encf8 = np.empty((128, NF8 * 256), np.float32)
    for j in range(NF8):
        cc = NBF + j
        for kh in range(2):
            encf8[:, j * 256 + kh * 128:j * 256 + (kh + 1) * 128] = \
                ET[kh * 128:(kh + 1) * 128, cc * 128:(cc + 1) * 128]
    encbf = encbf.astype(bf).copy()
    encf8 = encf8.astype(e4).copy()
    encb_v = np.asarray(ff_enc_b, np.float32)[perm]
    l2vc = np.asarray(l2_w, np.float32)[0].reshape(2, 128).T.copy()
    l1wc = t['l1w'].reshape(2, 128).T.copy()
    kappa = float(l2row @ np.asarray(l1_w, np.float64)[:, 0])
    c0 = float(l2row @ np.asarray(ff_dec_b, np.float64)
               + np.asarray(l2_b, np.float64)[0]
               + l2row @ np.asarray(l1_b, np.float64))
    kap = np.array([[kappa, c0]], np.float32)
    kapb = np.array([[kappa]], np.float32).astype(bf)

    flags = dict(
        has_yc=t['yc'] is not None,
        has_encb=bool(np.any(encb_v != 0)),
        has_c0=(c0 != 0.0),
        has_l1b=bool(np.any(t['l1b'] != 0)),
        tab_offs=t['tab_offs'],
    )
    nc = build_program(flags)

    shared = dict(tabs=t['tabs'], rbc=t['rbc'], wc=t['wc'],
                  dl1row=t['dl1row'], encbf=encbf, encf8=encf8,
                  vpk=vpk, l2v=l2vc, l1wc=l1wc, kap=kap, kapb=kapb)
    if flags['has_encb']:
        shared['encb'] = encb_v.reshape(NCC, 128).T.copy()
    if flags['has_l1b']:
        shared['l1bc'] = t['l1b'].reshape(2, 128).T.copy()
    if flags['has_yc']:
        shared['yc'] = t['yc']
    in_maps = []
    for b in range(B):
        xb = np.ascontiguousarray(x[b, :, 0]).astype(bf)[None, :]  # [1, L]
        m = dict(shared)
        m['xbs'] = xb
        in_maps.append(m)

    res = run_bass_kernel_spmd(nc, in_maps, list(range(B)))
    outs = [res.results[b]["out"][0][:, None] for b in range(B)]
    return np.stack(outs).astype(np.float32)


if __name__ == "__main__":
    pass



# revision 41
# speedup vs baseline: 1.1729x; 1.1729x over previous
"""Trainium2 Bass kernel for the CustomS5Block problem.

Strategy
--------
Data-parallel: batch 8 -> one batch element per NeuronCore.

Math: l1 has input dim 1, so u[l,h] = x[l]*l1w[h] (+l1_b) and
Bu[l,p] = x[l]*bb[p] with bb = B_bar @ l1w.  The diagonal S5 scan with
constant coefficient lam_bar = r*e^{i*phi} reduces to exponential
filters of the scalar signal x:

    K[l,p] = sum_{j<=l} r^{l-j} e^{i(l-j)phi} x[j]
    xs     = bb * K                       (folded into C on the host)

Rotation decomposition (exact, numerically stable):
    Sc[l,p] = r*Sc[l-1,p] + cos(l*phi_p)*x[l]     (tensor_tensor_scan)
    Ss[l,p] = r*Ss[l-1,p] + sin(l*phi_p)*x[l]
    Kr = ct*Sc + st*Ss ;  Ki = st*Sc - ct*Ss      (DVE elementwise)

Key algebraic fold: h2 = dec@f + dec_b + h1 is consumed ONLY by the
l2 head, so  out = l2@h2 + l2_b collapses to

    out[l] = v.f[:,l] + l2.th[:,l] + (l2.l1w)*x[l] + c0
    v  = l2 @ dec            (host, [CE])
    c0 = l2.dec_b + l2_b + l2.l1_b

which deletes the entire [CE->H] dec GEMM (2.7 GMAC/core) and h2.

Engines: PE does C-proj (bf16) + enc (f32r) + v-dot/l2th (f32r);
DVE does d1-muls (bf16 2x), scans (fp32), recombination; GpSimd does
so (=y+D*l1w*x), h1 (=th+l1w*x), orow (=psum+l2.l1w*x); ACT does the
tanh's (eps merged [128, 2*lc] across 2 PSUM banks).

Software pipeline: part_a two chunks ahead (DVE), C-projection one
chunk ahead (PE), v-dot tail + output deferred past the next
C-projection.  First chunks are smaller (256) to shorten the fill.
"""
import numpy as np
import ml_dtypes

import concourse.bass as bass
import concourse.tile as tile
from concourse import mybir
from concourse.bass_utils import run_bass_kernel_spmd

dt = mybir.dt
AF = mybir.ActivationFunctionType
OP = mybir.AluOpType

L = 4096
H = 256             # model width (2 tiles of 128)
P = 256             # state dim (2 tiles of 128)
CE = 2560           # 10*H
NCC = CE // 128     # 20
NG = NCC // 2       # 10 cc-pairs per chunk
NF8 = 10            # cc blocks (smallest |v|, permuted last) in fp8
NBF = NCC - NF8     # bf16 cc blocks

# chunk schedule: smaller leading chunks shorten the DVE fill
CHUNKS = [256, 256, 512, 512, 512, 512, 512, 512, 512]
assert sum(CHUNKS) == L
NCH = len(CHUNKS)
LCMAX = max(CHUNKS)

_ws_ctr = [0]


def _split_multi_waits(nc, max_waits=1):
    """walrus encodes at most one sync wait per compute instruction;
    hoist extras onto single-wait EventSemaphore ops on the same engine."""
    for func in nc.m.functions:
        for blk in func.blocks:
            new_insts = []
            for inst in blk.instructions:
                si = inst.sync_info
                if si is not None and len(si.on_wait) > max_waits:
                    waits = list(si.on_wait)
                    extra, keep = waits[:-max_waits], waits[-max_waits:]
                    for w in extra:
                        _ws_ctr[0] += 1
                        ev = mybir.InstEventSemaphore(
                            name=f"WSPLIT-{_ws_ctr[0]}", ins=[], outs=[],
                            engine=inst.engine)
                        ev.sync_info = mybir.SyncInfo(on_wait=[w], on_update=[])
                        new_insts.append(ev)
                    inst.sync_info = mybir.SyncInfo(
                        on_wait=keep, on_update=list(si.on_update))
                new_insts.append(inst)
            blk.instructions = new_insts
    return nc


def derive_host_tables(l1_w, l1_b, lam_re, lam_im, B_re, B_im, C_re, C_im,
                       D, log_step):
    """Parameter-only precompute (no dependence on x)."""
    l1w = np.asarray(l1_w, np.float32)[:, 0]
    l1b = np.asarray(l1_b, np.float32)
    lam = (np.asarray(lam_re, np.float32)
           + 1j * np.asarray(lam_im, np.float32)).astype(np.complex64)
    step = np.exp(np.asarray(log_step, np.float32)).astype(np.complex64)
    lam_bar = np.exp(lam * step)                       # complex64 [P]
    Bm = (np.asarray(B_re, np.float32)
          + 1j * np.asarray(B_im, np.float32)).astype(np.complex64)
    B_bar = ((lam_bar - 1.0) / lam)[:, None] * Bm      # [P, H]
    bb = B_bar @ l1w.astype(np.complex64)              # [P]

    r = np.abs(lam_bar).astype(np.float64)
    phi = np.angle(lam_bar).astype(np.float64)
    ls = np.arange(L, dtype=np.float64)
    ang = ls[None, :] * phi[:, None]                   # [P, L]
    ct = np.cos(ang).astype(np.float32)
    st = np.sin(ang).astype(np.float32)
    r32 = r.astype(np.float32)

    Cm = (np.asarray(C_re, np.float32)
          + 1j * np.asarray(C_im, np.float32)).astype(np.complex64)
    Ct = Cm * bb[None, :]                              # [H, P]
    Wr = (2.0 * Ct.real).T.astype(np.float32).copy()   # [P, H]
    Wi = (-2.0 * Ct.imag).T.astype(np.float32).copy()  # [P, H]
    dl1 = (np.asarray(D, np.float32) * l1w).astype(np.float32)

    # scan-side correction for nonzero l1_b: Bu gains the constant
    # bbb[p] = B_bar @ l1_b, whose scan is a closed-form geometric sum.
    if np.any(l1b != 0):
        bbb = (B_bar @ l1b.astype(np.complex64)).astype(np.complex128)
        lb = lam_bar.astype(np.complex128)
        pw = np.empty((P, L), np.complex128)
        acc = np.ones(P, np.complex128)
        for j in range(L):
            pw[:, j] = acc
            acc = acc * lb
        g = np.cumsum(pw, axis=1)
        xs_c = bbb[:, None] * g
        yc = 2.0 * np.real(Cm.astype(np.complex128) @ xs_c)  # [H, L]
        yc = yc.astype(np.float32)
    else:
        yc = None

    # host-side packing (bf16 tables):
    # tabs[row, chunk-offsets]: per chunk k and kp: [ct | st] of [128, lc]
    bf = ml_dtypes.bfloat16
    tabs = np.empty((128, 4 * L), bf)
    offs = []
    off = 0
    lo = 0
    for lc in CHUNKS:
        offs.append(off)
        for kp in range(2):
            tabs[:, off:off + lc] = ct[kp * 128:(kp + 1) * 128,
                                       lo:lo + lc].astype(bf)
            off += lc
            tabs[:, off:off + lc] = st[kp * 128:(kp + 1) * 128,
                                       lo:lo + lc].astype(bf)
            off += lc
        lo += lc
    rbc = np.repeat(r32[:, None], LCMAX, axis=1).copy()   # [P, LCMAX]
    # wc blocks: (term t in {r,i}, kp, hh) -> [128,128]
    wc = np.empty((128, 1024), bf)
    for t, W in enumerate((Wr, Wi)):
        for kp in range(2):
            for hh in range(2):
                blkidx = (t * 2 + kp) * 2 + hh
                wc[:, blkidx * 128:(blkidx + 1) * 128] = \
                    W[kp * 128:(kp + 1) * 128,
                      hh * 128:(hh + 1) * 128].astype(bf)
    return dict(tabs=tabs, tab_offs=offs, rbc=rbc, wc=wc,
                dl1row=dl1[None, :].astype(bf).copy(), l1w=l1w,
                l1b=l1b, yc=yc)


def build_program(flags):
    has_yc = flags['has_yc']
    has_encb = flags['has_encb']
    has_c0 = flags['has_c0']
    has_l1b = flags['has_l1b']
    tab_offs = flags['tab_offs']

    nc = bass.Bass("TRN2", target_bir_lowering=False, debug=False,
                   num_devices=8)
    f32, f32r, bf16 = dt.float32, dt.float32r, dt.bfloat16
    f8 = dt.float8e4
    PM = mybir.MatmulPerfMode

    xbs = nc.dram_tensor("xbs", [1, L], bf16, kind="ExternalInput")
    tabs = nc.dram_tensor("tabs", [128, 4 * L], bf16, kind="ExternalInput")
    rbc = nc.dram_tensor("rbc", [P, LCMAX], f32, kind="ExternalInput")
    wc = nc.dram_tensor("wc", [128, 1024], bf16, kind="ExternalInput")
    dl1row = nc.dram_tensor("dl1row", [1, H], bf16, kind="ExternalInput")
    encbf = nc.dram_tensor("encbf", [128, NBF * 256], bf16,
                           kind="ExternalInput")
    encf8 = nc.dram_tensor("encf8", [128, NF8 * 256], f8,
                           kind="ExternalInput")
    encb = nc.dram_tensor("encb", [128, NCC], f32, kind="ExternalInput") \
        if has_encb else None
    vpk = nc.dram_tensor("vpk", [128, NCC], f32r, kind="ExternalInput")
    l2v = nc.dram_tensor("l2v", [128, 2], f32r, kind="ExternalInput")
    l1wc = nc.dram_tensor("l1wc", [128, 2], f32, kind="ExternalInput")
    l1bc = nc.dram_tensor("l1bc", [128, 2], f32, kind="ExternalInput") \
        if has_l1b else None
    kap = nc.dram_tensor("kap", [1, 2], f32, kind="ExternalInput")
    kapb = nc.dram_tensor("kapb", [1, 1], bf16, kind="ExternalInput")
    ycd = nc.dram_tensor("yc", [H, L], f32, kind="ExternalInput") \
        if has_yc else None
    out = nc.dram_tensor("out", [1, L], f32, kind="ExternalOutput")

    los = np.cumsum([0] + CHUNKS).tolist()

    with tile.TileContext(nc) as tc:
        with tc.tile_pool(name="const", bufs=1) as cpool, \
             tc.tile_pool(name="stream", bufs=3) as spool, \
             tc.tile_pool(name="xpool", bufs=4) as xpool, \
             tc.tile_pool(name="work", bufs=2) as wpool, \
             tc.tile_pool(name="fpool", bufs=4) as fpool, \
             tc.tile_pool(name="ps_y", bufs=2, space="PSUM") as ps_y, \
             tc.tile_pool(name="ps_e", bufs=2, space="PSUM") as ps_e, \
             tc.tile_pool(name="ps_o", bufs=2, space="PSUM") as ps_o:

            # ---------- small constants (first: unblock the scan) ----------
            wc_sb = cpool.tile([128, 1024], bf16)
            nc.gpsimd.dma_start(out=wc_sb[:], in_=wc[:])
            rbc_sb = cpool.tile([128, 2 * LCMAX], f32)
            nc.gpsimd.dma_start(out=rbc_sb[:, 0:LCMAX], in_=rbc[0:128, :])
            nc.gpsimd.dma_start(out=rbc_sb[:, LCMAX:2 * LCMAX],
                                in_=rbc[128:256, :])
            dl1_sb = cpool.tile([1, H], bf16)
            nc.gpsimd.dma_start(out=dl1_sb[:], in_=dl1row[:])
            l1w_sb = cpool.tile([128, 2], f32)
            nc.gpsimd.dma_start(out=l1w_sb[:], in_=l1wc[:])
            v_sb = cpool.tile([128, NCC], f32r)
            nc.gpsimd.dma_start(out=v_sb[:], in_=vpk[:])
            l2v_sb = cpool.tile([128, 2], f32r)
            nc.gpsimd.dma_start(out=l2v_sb[:], in_=l2v[:])
            kap_sb = cpool.tile([1, 2], f32)
            nc.gpsimd.dma_start(out=kap_sb[:], in_=kap[:])
            kapb_sb = cpool.tile([1, 1], bf16)
            nc.gpsimd.dma_start(out=kapb_sb[:], in_=kapb[:])
            if has_encb:
                encb_sb = cpool.tile([128, NCC], f32)
                nc.gpsimd.dma_start(out=encb_sb[:], in_=encb[:])
            if has_l1b:
                l1b_sb = cpool.tile([128, 2], f32)
                nc.gpsimd.dma_start(out=l1b_sb[:], in_=l1bc[:])

            # PE warm-up tiles: memset, no DMA dependency
            wa = cpool.tile([128, 128], f32r)
            nc.vector.memset(wa[:].bitcast(f32), 0.0)
            wb = cpool.tile([128, 512], f32r)
            nc.vector.memset(wb[:].bitcast(f32), 0.0)

            def stream_chunk(i):
                lo, lc = los[i], CHUNKS[i]
                xb = xpool.tile([128, 2 * LCMAX], bf16, tag="xb",
                                name=f"xb_{i}")
                nc.sync.dma_start(
                    out=xb[:, 0:lc],
                    in_=xbs[0:1, lo:lo + lc].broadcast_to([128, lc]))
                nc.sync.dma_start(
                    out=xb[:, lc:2 * lc],
                    in_=xbs[0:1, lo:lo + lc].broadcast_to([128, lc]))
                tab_t = []
                for kp in range(2):
                    t = spool.tile([128, 2 * LCMAX], bf16, tag=f"tab{kp}",
                                   name=f"tab{kp}_{i}")
                    off = tab_offs[i] + kp * 2 * CHUNKS[i]
                    nc.sync.dma_start(out=t[:, 0:2 * lc],
                                      in_=tabs[:, off:off + 2 * lc])
                    tab_t.append(t)
                yc_t = None
                if has_yc:
                    yc_t = spool.tile([128, 2 * LCMAX], f32, tag="yc",
                                      name=f"yc_{i}")
                    nc.sync.dma_start(
                        out=yc_t[:, 0:2 * lc].rearrange(
                            "p (hh l) -> p hh l", hh=2),
                        in_=ycd.ap().rearrange("(hh p) l -> p hh l", p=128)
                        [:, :, lo:lo + lc])
                return tab_t, xb, yc_t

            # warm-up helper: matmul junk to keep the PE HAM un-throttled
            warm_ctr = [0]

            def warm(n, moving=None, mlen=512):
                for _ in range(n):
                    warm_ctr[0] += 1
                    wps = ps_e.tile([128, 2 * LCMAX], f32, tag="e",
                                    name=f"warm{warm_ctr[0]}")
                    mv = wb[:, 0:mlen] if moving is None else moving
                    nc.tensor.matmul(wps[:, 0:mlen], wa[:], mv,
                                     start=True, stop=True)

            prev_sc = [None, None]
            prev_ss = [None, None]
            prev_lc = [0]

            def part_a_d1(i, streams):
                """d1 muls for chunk i (DVE): [d1c|d1s] in one op per
                kp against the twice-replicated x tile."""
                lc = CHUNKS[i]
                tab_t, xb, yc_t = streams
                d1 = []
                for kp in range(2):
                    d1cs = wpool.tile([128, 2 * LCMAX], bf16,
                                      tag=f"d1cs{kp}", name=f"d1cs{kp}_{i}")
                    nc.vector.tensor_mul(d1cs[:, 0:2 * lc],
                                         tab_t[kp][:, 0:2 * lc],
                                         xb[:, 0:2 * lc])
                    d1.append(d1cs)
                return d1

            def part_a(i, streams, d1=None):
                """scans + recombination for chunk i (DVE only)."""
                lc = CHUNKS[i]
                tab_t, xb, yc_t = streams
                if d1 is None:
                    d1 = part_a_d1(i, streams)
                scss_t = []
                for kp in range(2):
                    d1cs = d1[kp]
                    r_ap = rbc_sb[:, kp * LCMAX:kp * LCMAX + lc]
                    scss = wpool.tile([128, 2 * LCMAX], f32, tag=f"scss{kp}",
                                      name=f"scss{kp}_{i}")
                    init_c = 0.0 if i == 0 else \
                        prev_sc[kp][:, prev_lc[0] - 1:prev_lc[0]]
                    nc.vector.tensor_tensor_scan(
                        scss[:, 0:lc], r_ap, d1cs[:, 0:lc], init_c,
                        OP.mult, OP.add)
                    init_s = 0.0 if i == 0 else \
                        prev_ss[kp][:, 2 * prev_lc[0] - 1:2 * prev_lc[0]]
                    nc.vector.tensor_tensor_scan(
                        scss[:, lc:2 * lc], r_ap, d1cs[:, lc:2 * lc], init_s,
                        OP.mult, OP.add)
                    scss_t.append(scss)
                prev_sc[:] = scss_t
                prev_ss[:] = scss_t
                prev_lc[0] = lc

                kr_t, ki_t = [], []
                for kp in range(2):
                    ct_ap = tab_t[kp][:, 0:lc]
                    st_ap = tab_t[kp][:, lc:2 * lc]
                    sc_ap = scss_t[kp][:, 0:lc]
                    ss_ap = scss_t[kp][:, lc:2 * lc]
                    # m1 = [ct*Sc | st*Ss] in one paired op, then
                    # Kr = ct*Sc + st*Ss ; Ki = st*Sc - ct*Ss
                    m1 = wpool.tile([128, 2 * LCMAX], bf16, tag=f"m1{kp}",
                                    name=f"m1{kp}_{i}")
                    nc.vector.tensor_mul(m1[:, 0:2 * lc],
                                         tab_t[kp][:, 0:2 * lc],
                                         scss_t[kp][:, 0:2 * lc])
                    kr = wpool.tile([128, LCMAX], bf16, tag=f"kr{kp}",
                                    name=f"kr{kp}_{i}")
                    nc.vector.tensor_add(kr[:, 0:lc], m1[:, 0:lc],
                                         m1[:, lc:2 * lc])
                    s2 = wpool.tile([128, LCMAX], bf16, tag=f"s2{kp}",
                                    name=f"s2{kp}_{i}")
                    ki = wpool.tile([128, LCMAX], bf16, tag=f"ki{kp}",
                                    name=f"ki{kp}_{i}")
                    nc.vector.tensor_mul(s2[:, 0:lc], st_ap, sc_ap)
                    nc.vector.tensor_mul(ki[:, 0:lc], ct_ap, ss_ap)
                    nc.vector.tensor_sub(ki[:, 0:lc], s2[:, 0:lc], ki[:, 0:lc])
                    kr_t.append(kr)
                    ki_t.append(ki)
                return kr_t, ki_t, xb, yc_t

            def cproj(i, pa):
                """C-projection (PE, with the D*l1w*x rank-1 term folded
                in as a K=1 matmul so th reads PSUM directly) + th (ACT)
                + h1 (DVE stt, bf16 + fp8 shadow at equal cost)."""
                lc = CHUNKS[i]
                kr_t, ki_t, xb, yc_t = pa
                th_t, h1_t = [], []
                h1f = wpool.tile([128, 2 * LCMAX], f8, tag="h1f",
                                 name=f"h1f_{i}") if NF8 > 0 else None
                for hh in range(2):
                    yps = ps_y.tile([128, LCMAX], f32, tag="y",
                                    name=f"y{hh}_{i}")
                    for mi, ks in enumerate((kr_t, ki_t)):
                        for kp in range(2):
                            blkidx = (mi * 2 + kp) * 2 + hh
                            nc.tensor.matmul(
                                yps[:, 0:lc],
                                wc_sb[:, blkidx * 128:(blkidx + 1) * 128],
                                ks[kp][:, 0:lc], start=(mi == 0 and kp == 0),
                                stop=False)
                    # D*l1w*x rank-1 term as a K=1 matmul
                    nc.tensor.matmul(
                        yps[:, 0:lc], dl1_sb[0:1, hh * 128:(hh + 1) * 128],
                        xb[0:1, 0:lc], start=False, stop=True)
                    if has_yc:
                        nc.vector.tensor_add(
                            yps[:, 0:lc], yps[:, 0:lc],
                            yc_t[:, hh * lc:(hh + 1) * lc])
                    th = wpool.tile([128, LCMAX], f32r, tag=f"th{hh}",
                                    name=f"th{hh}_{i}")
                    nc.scalar.activation(th[:, 0:lc],
                                         yps[:, 0:lc], AF.Tanh)
                    h1 = wpool.tile([128, LCMAX], bf16, tag=f"h1{hh}",
                                    name=f"h1{hh}_{i}")
                    nc.vector.scalar_tensor_tensor(
                        h1[:, 0:lc], xb[:, 0:lc],
                        l1w_sb[:, hh:hh + 1], th[:, 0:lc].bitcast(f32),
                        OP.mult, OP.add)
                    if has_l1b:
                        nc.vector.tensor_scalar_add(
                            h1[:, 0:lc], h1[:, 0:lc],
                            l1b_sb[:, hh:hh + 1])
                    if h1f is not None:
                        if has_l1b:
                            nc.vector.tensor_copy(
                                h1f[:, hh * lc:hh * lc + lc], h1[:, 0:lc])
                        else:
                            nc.vector.scalar_tensor_tensor(
                                h1f[:, hh * lc:hh * lc + lc], xb[:, 0:lc],
                                l1w_sb[:, hh:hh + 1],
                                th[:, 0:lc].bitcast(f32),
                                OP.mult, OP.add)
                    th_t.append(th)
                    h1_t.append(h1)
                return th_t, h1_t, h1f, xb

            # ------------- prologue -------------
            streams = [None] * NCH
            pa = [None] * NCH
            streams[0] = stream_chunk(0)
            streams[1] = stream_chunk(1)
            warm(12)

            # large resident weights
            encbf_sb = cpool.tile([128, NBF * 256], bf16)
            nc.gpsimd.dma_start(out=encbf_sb[:], in_=encbf[:])
            encf8_sb = cpool.tile([128, NF8 * 256], f8)
            nc.gpsimd.dma_start(out=encf8_sb[:], in_=encf8[:])

            warm(6)
            pa[0] = part_a(0, streams[0])
            warm(10)
            pa[1] = part_a(1, streams[1])
            warm(10)

            cp = [None] * NCH
            cp[0] = cproj(0, pa[0])

            pending_tail = None

            def mlp(i, orow, mid=None):
                lo, lc = los[i], CHUNKS[i]
                th_t, h1_t, h1f, xb = cp[i]
                for hh in range(2):
                    nc.tensor.matmul(orow[0:1, 0:lc],
                                     l2v_sb[:, hh:hh + 1],
                                     th_t[hh][:, 0:lc],
                                     start=(hh == 0), stop=False)
                # (l2.l1w)*x rank-1 term as a K=1 matmul
                nc.tensor.matmul(orow[0:1, 0:lc], kapb_sb[0:1, 0:1],
                                 xb[0:1, 0:lc], start=False, stop=False)
                h1f2 = None if h1f is None else \
                    h1f[:, 0:2 * lc].rearrange("p (two l) -> p two l", two=2)
                f_prev = None
                for g in range(NG):
                    eps = ps_e.tile([128, 2 * LCMAX], f32, tag="e",
                                    name=f"eps{g}_{i}")
                    for half in range(2):
                        cc = 2 * g + half
                        if cc >= NBF:
                            j = cc - NBF
                            nc.tensor.matmul(
                                eps[:, half * lc:half * lc + lc],
                                encf8_sb[:, j * 256:(j + 1) * 256].rearrange(
                                    "p (two m) -> p two m", two=2),
                                h1f2, start=True, stop=True,
                                perf_mode=PM.DoubleRow)
                            continue
                        for kh in range(2):
                            nc.tensor.matmul(
                                eps[:, half * lc:half * lc + lc],
                                encbf_sb[:, cc * 256 + kh * 128:
                                         cc * 256 + (kh + 1) * 128],
                                h1_t[kh][:, 0:lc],
                                start=(kh == 0), stop=(kh == 1))
                    f_t = fpool.tile([128, 2 * LCMAX], f32r, tag="f",
                                     name=f"f{g}_{i}")
                    if has_encb:
                        for half in range(2):
                            cc = 2 * g + half
                            nc.scalar.activation(
                                f_t[:, half * lc:half * lc + lc],
                                eps[:, half * lc:half * lc + lc], AF.Tanh,
                                bias=encb_sb[:, cc:cc + 1])
                    else:
                        nc.scalar.activation(f_t[:, 0:2 * lc],
                                             eps[:, 0:2 * lc], AF.Tanh)
                    if f_prev is not None:
                        gp = g - 1
                        for half in range(2):
                            cc = 2 * gp + half
                            nc.tensor.matmul(
                                orow[0:1, 0:lc], v_sb[:, cc:cc + 1],
                                f_prev[:, half * lc:half * lc + lc],
                                start=False, stop=False)
                    f_prev = f_t
                    if g == 2 and mid is not None:
                        mid()
                return f_prev

            for i in range(NCH):
                lo, lc = los[i], CHUNKS[i]
                xb = cp[i][3]

                # streams two chunks ahead; d1 muls go on the DVE queue
                # before h1 so the scan pipeline isn't head-of-line blocked
                d1_next = None
                if i + 2 < NCH:
                    streams[i + 2] = stream_chunk(i + 2)
                    d1_next = part_a_d1(i + 2, streams[i + 2])

                orow = ps_o.tile([1, LCMAX], f32, tag="or", name=f"or_{i}")

                # During the fill the DVE scan pipeline lags the PE, so
                # the next C-projection would head-of-line-block the PE;
                # emit it after this chunk's MLP instead.  From chunk 3
                # on (DVE caught up) emit it first (steady state).  The
                # deferred v-dot tail goes after the next C-projection so
                # the PE never waits on the last tanh group.
                early = i < 3
                if early:
                    if pending_tail is not None:
                        pending_tail()
                        pending_tail = None
                    f_prev = mlp(i, orow)
                    if i + 1 < NCH:
                        cp[i + 1] = cproj(i + 1, pa[i + 1])
                else:
                    if i + 1 < NCH:
                        cp[i + 1] = cproj(i + 1, pa[i + 1])
                    if pending_tail is not None:
                        pending_tail()
                        pending_tail = None
                    f_prev = mlp(i, orow)

                if i + 2 < NCH:
                    pa[i + 2] = part_a(i + 2, streams[i + 2], d1=d1_next)

                def tail(i=i, lo=lo, lc=lc, orow=orow, f_prev=f_prev):
                    for half in range(2):
                        cc = 2 * (NG - 1) + half
                        nc.tensor.matmul(
                            orow[0:1, 0:lc], v_sb[:, cc:cc + 1],
                            f_prev[:, half * lc:half * lc + lc],
                            start=False, stop=(half == 1))
                    og = wpool.tile([1, LCMAX], f32, tag="og",
                                    name=f"og_{i}")
                    if has_c0:
                        nc.scalar.activation(og[0:1, 0:lc], orow[0:1, 0:lc],
                                             AF.Identity,
                                             bias=kap_sb[0:1, 1:2])
                    else:
                        nc.scalar.activation(og[0:1, 0:lc], orow[0:1, 0:lc],
                                             AF.Identity)
                    nc.sync.dma_start(out=out[0:1, lo:lo + lc],
                                      in_=og[0:1, 0:lc])
                pending_tail = tail

            pending_tail()

    _split_multi_waits(nc)
    return nc


def kernel(x, l1_w, l1_b, lam_re, lam_im, B_re, B_im, C_re, C_im, D,
           log_step, ff_enc_w, ff_enc_b, ff_dec_w, ff_dec_b, l2_w, l2_b):
    bf = ml_dtypes.bfloat16
    e4 = ml_dtypes.float8_e4m3fn
    x = np.asarray(x, np.float32)
    B = x.shape[0]
    t = derive_host_tables(l1_w, l1_b, lam_re, lam_im, B_re, B_im,
                           C_re, C_im, D, log_step)

    l2row = np.asarray(l2_w, np.float64)[0]            # [H]
    dec_w = np.asarray(ff_dec_w, np.float64)           # [H, CE]
    v = (l2row @ dec_w).astype(np.float32)             # [CE]

    # permute MLP channels by |v|: the NF8*128 smallest go to the fp8
    # cc blocks (placed last), so fp8 quantization noise lands on the
    # channels with the least output weight.
    perm = np.argsort(np.abs(v))                       # ascending |v|
    perm = np.concatenate([perm[NF8 * 128:], perm[:NF8 * 128]])
    v_p = v[perm]
    vpk = v_p.reshape(NCC, 128).T.copy()               # [128, NCC]

    enc_w = np.asarray(ff_enc_w, np.float32)[perm]     # [CE, H] permuted
    ET = enc_w.T                                       # [H, CE]
    encbf = np.empty((128, NBF * 256), np.float32)
    for cc in range(NBF):
        for kh in range(2):
            encbf[:, cc * 256 + kh * 128:cc * 256 + (kh + 1) * 128] = \
                ET[kh * 128:(kh + 1) * 128, cc * 128:(cc + 1) * 128]
    encf8 = np.empty((128, NF8 * 256), np.float32)
    for j in range(NF8):
        cc = NBF + j
        for kh in range(2):
            encf8[:, j * 256 + kh * 128:j * 256 + (kh + 1) * 128] = \
                ET[kh * 128:(kh + 1) * 128, cc * 128:(cc + 1) * 128]
    encbf = encbf.astype(bf).copy()
    encf8 = encf8.astype(e4).copy()
    encb_v = np.asarray(ff_enc_b, np.float32)[perm]
    l2vc = np.asarray(l2_w, np.float32)[0].reshape(2, 128).T.copy()
    l1wc = t['l1w'].reshape(2, 128).T.copy()
    kappa = float(l2row @ np.asarray(l1_w, np.float64)[:, 0])
    c0 = float(l2row @ np.asarray(ff_dec_b, np.float64)
               + np.asarray(l2_b, np.float64)[0]
               + l2row @ np.asarray(l1_b, np.float64))
    kap = np.array([[kappa, c0]], np.float32)
    kapb = np.array([[kappa]], np.float32).astype(bf)

    flags = dict(
        has_yc=t['yc'] is not None,
        has_encb=bool(np.any(encb_v != 0)),
        has_c0=(c0 != 0.0),
        has_l1b=bool(np.any(t['l1b'] != 0)),
        tab_offs=t['tab_offs'],
    )
    nc = build_program(flags)

    shared = dict(tabs=t['tabs'], rbc=t['rbc'], wc=t['wc'],
                  dl1row=t['dl1row'], encbf=encbf, encf8=encf8,
                  vpk=vpk, l2v=l2vc, l1wc=l1wc, kap=kap, kapb=kapb)
    if flags['has_encb']:
        shared['encb'] = encb_v.reshape(NCC, 128).T.copy()
    if flags['has_l1b']:
        shared['l1bc'] = t['l1b'].reshape(2, 128).T.copy()
    if flags['has_yc']:
        shared['yc'] = t['yc']
    in_maps = []
    for b in range(B):
        xb = np.ascontiguousarray(x[b, :, 0]).astype(bf)[None, :]  # [1, L]
        m = dict(shared)
        m['xbs'] = xb
        in_maps.append(m)

    res = run_bass_kernel_spmd(nc, in_maps, list(range(B)))
    outs = [res.results[b]["out"][0][:, None] for b in range(B)]
    return np.stack(outs).astype(np.float32)


if __name__ == "__main__":
    pass



# revision 42
# speedup vs baseline: 1.1774x; 1.0038x over previous
"""Trainium2 Bass kernel for the CustomS5Block problem.

Strategy
--------
Data-parallel: batch 8 -> one batch element per NeuronCore.

Math: l1 has input dim 1, so u[l,h] = x[l]*l1w[h] (+l1_b) and
Bu[l,p] = x[l]*bb[p] with bb = B_bar @ l1w.  The diagonal S5 scan with
constant coefficient lam_bar = r*e^{i*phi} reduces to exponential
filters of the scalar signal x:

    K[l,p] = sum_{j<=l} r^{l-j} e^{i(l-j)phi} x[j]
    xs     = bb * K                       (folded into C on the host)

Rotation decomposition (exact, numerically stable):
    Sc[l,p] = r*Sc[l-1,p] + cos(l*phi_p)*x[l]     (tensor_tensor_scan)
    Ss[l,p] = r*Ss[l-1,p] + sin(l*phi_p)*x[l]
    Kr = ct*Sc + st*Ss ;  Ki = st*Sc - ct*Ss      (DVE elementwise)

Key algebraic fold: h2 = dec@f + dec_b + h1 is consumed ONLY by the
l2 head, so  out = l2@h2 + l2_b collapses to

    out[l] = v.f[:,l] + l2.th[:,l] + (l2.l1w)*x[l] + c0
    v  = l2 @ dec            (host, [CE])
    c0 = l2.dec_b + l2_b + l2.l1_b

which deletes the entire [CE->H] dec GEMM (2.7 GMAC/core) and h2.

Engines: PE does C-proj (bf16) + enc (f32r) + v-dot/l2th (f32r);
DVE does d1-muls (bf16 2x), scans (fp32), recombination; GpSimd does
so (=y+D*l1w*x), h1 (=th+l1w*x), orow (=psum+l2.l1w*x); ACT does the
tanh's (eps merged [128, 2*lc] across 2 PSUM banks).

Software pipeline: part_a two chunks ahead (DVE), C-projection one
chunk ahead (PE), v-dot tail + output deferred past the next
C-projection.  First chunks are smaller (256) to shorten the fill.
"""
import numpy as np
import ml_dtypes

import concourse.bass as bass
import concourse.tile as tile
from concourse import mybir
from concourse.bass_utils import run_bass_kernel_spmd

dt = mybir.dt
AF = mybir.ActivationFunctionType
OP = mybir.AluOpType

L = 4096
H = 256             # model width (2 tiles of 128)
P = 256             # state dim (2 tiles of 128)
CE = 2560           # 10*H
NCC = CE // 128     # 20
NG = NCC // 2       # 10 cc-pairs per chunk
NF8 = 11            # cc blocks (smallest |v|, permuted last) in fp8
NBF = NCC - NF8     # bf16 cc blocks

# chunk schedule: smaller leading chunks shorten the DVE fill
CHUNKS = [256, 256, 512, 512, 512, 512, 512, 512, 512]
assert sum(CHUNKS) == L
NCH = len(CHUNKS)
LCMAX = max(CHUNKS)

_ws_ctr = [0]


def _split_multi_waits(nc, max_waits=1):
    """walrus encodes at most one sync wait per compute instruction;
    hoist extras onto single-wait EventSemaphore ops on the same engine."""
    for func in nc.m.functions:
        for blk in func.blocks:
            new_insts = []
            for inst in blk.instructions:
                si = inst.sync_info
                if si is not None and len(si.on_wait) > max_waits:
                    waits = list(si.on_wait)
                    extra, keep = waits[:-max_waits], waits[-max_waits:]
                    for w in extra:
                        _ws_ctr[0] += 1
                        ev = mybir.InstEventSemaphore(
                            name=f"WSPLIT-{_ws_ctr[0]}", ins=[], outs=[],
                            engine=inst.engine)
                        ev.sync_info = mybir.SyncInfo(on_wait=[w], on_update=[])
                        new_insts.append(ev)
                    inst.sync_info = mybir.SyncInfo(
                        on_wait=keep, on_update=list(si.on_update))
                new_insts.append(inst)
            blk.instructions = new_insts
    return nc


def derive_host_tables(l1_w, l1_b, lam_re, lam_im, B_re, B_im, C_re, C_im,
                       D, log_step):
    """Parameter-only precompute (no dependence on x)."""
    l1w = np.asarray(l1_w, np.float32)[:, 0]
    l1b = np.asarray(l1_b, np.float32)
    lam = (np.asarray(lam_re, np.float32)
           + 1j * np.asarray(lam_im, np.float32)).astype(np.complex64)
    step = np.exp(np.asarray(log_step, np.float32)).astype(np.complex64)
    lam_bar = np.exp(lam * step)                       # complex64 [P]
    Bm = (np.asarray(B_re, np.float32)
          + 1j * np.asarray(B_im, np.float32)).astype(np.complex64)
    B_bar = ((lam_bar - 1.0) / lam)[:, None] * Bm      # [P, H]
    bb = B_bar @ l1w.astype(np.complex64)              # [P]

    r = np.abs(lam_bar).astype(np.float64)
    phi = np.angle(lam_bar).astype(np.float64)
    ls = np.arange(L, dtype=np.float64)
    ang = ls[None, :] * phi[:, None]                   # [P, L]
    ct = np.cos(ang).astype(np.float32)
    st = np.sin(ang).astype(np.float32)
    r32 = r.astype(np.float32)

    Cm = (np.asarray(C_re, np.float32)
          + 1j * np.asarray(C_im, np.float32)).astype(np.complex64)
    Ct = Cm * bb[None, :]                              # [H, P]
    Wr = (2.0 * Ct.real).T.astype(np.float32).copy()   # [P, H]
    Wi = (-2.0 * Ct.imag).T.astype(np.float32).copy()  # [P, H]
    dl1 = (np.asarray(D, np.float32) * l1w).astype(np.float32)

    # scan-side correction for nonzero l1_b: Bu gains the constant
    # bbb[p] = B_bar @ l1_b, whose scan is a closed-form geometric sum.
    if np.any(l1b != 0):
        bbb = (B_bar @ l1b.astype(np.complex64)).astype(np.complex128)
        lb = lam_bar.astype(np.complex128)
        pw = np.empty((P, L), np.complex128)
        acc = np.ones(P, np.complex128)
        for j in range(L):
            pw[:, j] = acc
            acc = acc * lb
        g = np.cumsum(pw, axis=1)
        xs_c = bbb[:, None] * g
        yc = 2.0 * np.real(Cm.astype(np.complex128) @ xs_c)  # [H, L]
        yc = yc.astype(np.float32)
    else:
        yc = None

    # host-side packing (bf16 tables):
    # tabs[row, chunk-offsets]: per chunk k and kp: [ct | st] of [128, lc]
    bf = ml_dtypes.bfloat16
    tabs = np.empty((128, 4 * L), bf)
    offs = []
    off = 0
    lo = 0
    for lc in CHUNKS:
        offs.append(off)
        for kp in range(2):
            tabs[:, off:off + lc] = ct[kp * 128:(kp + 1) * 128,
                                       lo:lo + lc].astype(bf)
            off += lc
            tabs[:, off:off + lc] = st[kp * 128:(kp + 1) * 128,
                                       lo:lo + lc].astype(bf)
            off += lc
        lo += lc
    rbc = np.repeat(r32[:, None], LCMAX, axis=1).copy()   # [P, LCMAX]
    # wc blocks: (term t in {r,i}, kp, hh) -> [128,128]
    wc = np.empty((128, 1024), bf)
    for t, W in enumerate((Wr, Wi)):
        for kp in range(2):
            for hh in range(2):
                blkidx = (t * 2 + kp) * 2 + hh
                wc[:, blkidx * 128:(blkidx + 1) * 128] = \
                    W[kp * 128:(kp + 1) * 128,
                      hh * 128:(hh + 1) * 128].astype(bf)
    return dict(tabs=tabs, tab_offs=offs, rbc=rbc, wc=wc,
                dl1row=dl1[None, :].astype(bf).copy(), l1w=l1w,
                l1b=l1b, yc=yc)


def build_program(flags):
    has_yc = flags['has_yc']
    has_encb = flags['has_encb']
    has_c0 = flags['has_c0']
    has_l1b = flags['has_l1b']
    tab_offs = flags['tab_offs']

    nc = bass.Bass("TRN2", target_bir_lowering=False, debug=False,
                   num_devices=8)
    f32, f32r, bf16 = dt.float32, dt.float32r, dt.bfloat16
    f8 = dt.float8e4
    PM = mybir.MatmulPerfMode

    xbs = nc.dram_tensor("xbs", [1, L], bf16, kind="ExternalInput")
    tabs = nc.dram_tensor("tabs", [128, 4 * L], bf16, kind="ExternalInput")
    rbc = nc.dram_tensor("rbc", [P, LCMAX], f32, kind="ExternalInput")
    wc = nc.dram_tensor("wc", [128, 1024], bf16, kind="ExternalInput")
    dl1row = nc.dram_tensor("dl1row", [1, H], bf16, kind="ExternalInput")
    encbf = nc.dram_tensor("encbf", [128, NBF * 256], bf16,
                           kind="ExternalInput")
    encf8 = nc.dram_tensor("encf8", [128, NF8 * 256], f8,
                           kind="ExternalInput")
    encb = nc.dram_tensor("encb", [128, NCC], f32, kind="ExternalInput") \
        if has_encb else None
    vpk = nc.dram_tensor("vpk", [128, NCC], f32r, kind="ExternalInput")
    l2v = nc.dram_tensor("l2v", [128, 2], f32r, kind="ExternalInput")
    l1wc = nc.dram_tensor("l1wc", [128, 2], f32, kind="ExternalInput")
    l1bc = nc.dram_tensor("l1bc", [128, 2], f32, kind="ExternalInput") \
        if has_l1b else None
    kap = nc.dram_tensor("kap", [1, 2], f32, kind="ExternalInput")
    kapb = nc.dram_tensor("kapb", [1, 1], bf16, kind="ExternalInput")
    ycd = nc.dram_tensor("yc", [H, L], f32, kind="ExternalInput") \
        if has_yc else None
    out = nc.dram_tensor("out", [1, L], f32, kind="ExternalOutput")

    los = np.cumsum([0] + CHUNKS).tolist()

    with tile.TileContext(nc) as tc:
        with tc.tile_pool(name="const", bufs=1) as cpool, \
             tc.tile_pool(name="stream", bufs=3) as spool, \
             tc.tile_pool(name="xpool", bufs=4) as xpool, \
             tc.tile_pool(name="work", bufs=2) as wpool, \
             tc.tile_pool(name="fpool", bufs=4) as fpool, \
             tc.tile_pool(name="ps_y", bufs=2, space="PSUM") as ps_y, \
             tc.tile_pool(name="ps_e", bufs=2, space="PSUM") as ps_e, \
             tc.tile_pool(name="ps_o", bufs=2, space="PSUM") as ps_o:

            # ---------- small constants (first: unblock the scan) ----------
            wc_sb = cpool.tile([128, 1024], bf16)
            nc.gpsimd.dma_start(out=wc_sb[:], in_=wc[:])
            rbc_sb = cpool.tile([128, 2 * LCMAX], f32)
            nc.gpsimd.dma_start(out=rbc_sb[:, 0:LCMAX], in_=rbc[0:128, :])
            nc.gpsimd.dma_start(out=rbc_sb[:, LCMAX:2 * LCMAX],
                                in_=rbc[128:256, :])
            dl1_sb = cpool.tile([1, H], bf16)
            nc.gpsimd.dma_start(out=dl1_sb[:], in_=dl1row[:])
            l1w_sb = cpool.tile([128, 2], f32)
            nc.gpsimd.dma_start(out=l1w_sb[:], in_=l1wc[:])
            v_sb = cpool.tile([128, NCC], f32r)
            nc.gpsimd.dma_start(out=v_sb[:], in_=vpk[:])
            l2v_sb = cpool.tile([128, 2], f32r)
            nc.gpsimd.dma_start(out=l2v_sb[:], in_=l2v[:])
            kap_sb = cpool.tile([1, 2], f32)
            nc.gpsimd.dma_start(out=kap_sb[:], in_=kap[:])
            kapb_sb = cpool.tile([1, 1], bf16)
            nc.gpsimd.dma_start(out=kapb_sb[:], in_=kapb[:])
            if has_encb:
                encb_sb = cpool.tile([128, NCC], f32)
                nc.gpsimd.dma_start(out=encb_sb[:], in_=encb[:])
            if has_l1b:
                l1b_sb = cpool.tile([128, 2], f32)
                nc.gpsimd.dma_start(out=l1b_sb[:], in_=l1bc[:])

            # PE warm-up tiles: memset, no DMA dependency
            wa = cpool.tile([128, 128], f32r)
            nc.vector.memset(wa[:].bitcast(f32), 0.0)
            wb = cpool.tile([128, 512], f32r)
            nc.vector.memset(wb[:].bitcast(f32), 0.0)

            def stream_chunk(i):
                lo, lc = los[i], CHUNKS[i]
                xb = xpool.tile([128, 2 * LCMAX], bf16, tag="xb",
                                name=f"xb_{i}")
                nc.sync.dma_start(
                    out=xb[:, 0:lc],
                    in_=xbs[0:1, lo:lo + lc].broadcast_to([128, lc]))
                nc.sync.dma_start(
                    out=xb[:, lc:2 * lc],
                    in_=xbs[0:1, lo:lo + lc].broadcast_to([128, lc]))
                tab_t = []
                for kp in range(2):
                    t = spool.tile([128, 2 * LCMAX], bf16, tag=f"tab{kp}",
                                   name=f"tab{kp}_{i}")
                    off = tab_offs[i] + kp * 2 * CHUNKS[i]
                    nc.sync.dma_start(out=t[:, 0:2 * lc],
                                      in_=tabs[:, off:off + 2 * lc])
                    tab_t.append(t)
                yc_t = None
                if has_yc:
                    yc_t = spool.tile([128, 2 * LCMAX], f32, tag="yc",
                                      name=f"yc_{i}")
                    nc.sync.dma_start(
                        out=yc_t[:, 0:2 * lc].rearrange(
                            "p (hh l) -> p hh l", hh=2),
                        in_=ycd.ap().rearrange("(hh p) l -> p hh l", p=128)
                        [:, :, lo:lo + lc])
                return tab_t, xb, yc_t

            # warm-up helper: matmul junk to keep the PE HAM un-throttled
            warm_ctr = [0]

            def warm(n, moving=None, mlen=512):
                for _ in range(n):
                    warm_ctr[0] += 1
                    wps = ps_e.tile([128, 2 * LCMAX], f32, tag="e",
                                    name=f"warm{warm_ctr[0]}")
                    mv = wb[:, 0:mlen] if moving is None else moving
                    nc.tensor.matmul(wps[:, 0:mlen], wa[:], mv,
                                     start=True, stop=True)

            prev_sc = [None, None]
            prev_ss = [None, None]
            prev_lc = [0]

            def part_a_d1(i, streams):
                """d1 muls for chunk i (DVE): [d1c|d1s] in one op per
                kp against the twice-replicated x tile."""
                lc = CHUNKS[i]
                tab_t, xb, yc_t = streams
                d1 = []
                for kp in range(2):
                    d1cs = wpool.tile([128, 2 * LCMAX], bf16,
                                      tag=f"d1cs{kp}", name=f"d1cs{kp}_{i}")
                    nc.vector.tensor_mul(d1cs[:, 0:2 * lc],
                                         tab_t[kp][:, 0:2 * lc],
                                         xb[:, 0:2 * lc])
                    d1.append(d1cs)
                return d1

            def part_a(i, streams, d1=None):
                """scans + recombination for chunk i (DVE only)."""
                lc = CHUNKS[i]
                tab_t, xb, yc_t = streams
                if d1 is None:
                    d1 = part_a_d1(i, streams)
                scss_t = []
                for kp in range(2):
                    d1cs = d1[kp]
                    r_ap = rbc_sb[:, kp * LCMAX:kp * LCMAX + lc]
                    scss = wpool.tile([128, 2 * LCMAX], f32, tag=f"scss{kp}",
                                      name=f"scss{kp}_{i}")
                    init_c = 0.0 if i == 0 else \
                        prev_sc[kp][:, prev_lc[0] - 1:prev_lc[0]]
                    nc.vector.tensor_tensor_scan(
                        scss[:, 0:lc], r_ap, d1cs[:, 0:lc], init_c,
                        OP.mult, OP.add)
                    init_s = 0.0 if i == 0 else \
                        prev_ss[kp][:, 2 * prev_lc[0] - 1:2 * prev_lc[0]]
                    nc.vector.tensor_tensor_scan(
                        scss[:, lc:2 * lc], r_ap, d1cs[:, lc:2 * lc], init_s,
                        OP.mult, OP.add)
                    scss_t.append(scss)
                prev_sc[:] = scss_t
                prev_ss[:] = scss_t
                prev_lc[0] = lc

                kr_t, ki_t = [], []
                for kp in range(2):
                    ct_ap = tab_t[kp][:, 0:lc]
                    st_ap = tab_t[kp][:, lc:2 * lc]
                    sc_ap = scss_t[kp][:, 0:lc]
                    ss_ap = scss_t[kp][:, lc:2 * lc]
                    # m1 = [ct*Sc | st*Ss] in one paired op, then
                    # Kr = ct*Sc + st*Ss ; Ki = st*Sc - ct*Ss
                    m1 = wpool.tile([128, 2 * LCMAX], bf16, tag=f"m1{kp}",
                                    name=f"m1{kp}_{i}")
                    nc.vector.tensor_mul(m1[:, 0:2 * lc],
                                         tab_t[kp][:, 0:2 * lc],
                                         scss_t[kp][:, 0:2 * lc])
                    kr = wpool.tile([128, LCMAX], bf16, tag=f"kr{kp}",
                                    name=f"kr{kp}_{i}")
                    nc.vector.tensor_add(kr[:, 0:lc], m1[:, 0:lc],
                                         m1[:, lc:2 * lc])
                    s2 = wpool.tile([128, LCMAX], bf16, tag=f"s2{kp}",
                                    name=f"s2{kp}_{i}")
                    ki = wpool.tile([128, LCMAX], bf16, tag=f"ki{kp}",
                                    name=f"ki{kp}_{i}")
                    nc.vector.tensor_mul(s2[:, 0:lc], st_ap, sc_ap)
                    nc.vector.tensor_mul(ki[:, 0:lc], ct_ap, ss_ap)
                    nc.vector.tensor_sub(ki[:, 0:lc], s2[:, 0:lc], ki[:, 0:lc])
                    kr_t.append(kr)
                    ki_t.append(ki)
                return kr_t, ki_t, xb, yc_t

            def cproj(i, pa):
                """C-projection (PE, with the D*l1w*x rank-1 term folded
                in as a K=1 matmul so th reads PSUM directly) + th (ACT)
                + h1 (DVE stt, bf16 + fp8 shadow at equal cost)."""
                lc = CHUNKS[i]
                kr_t, ki_t, xb, yc_t = pa
                th_t, h1_t = [], []
                h1f = wpool.tile([128, 2 * LCMAX], f8, tag="h1f",
                                 name=f"h1f_{i}") if NF8 > 0 else None
                for hh in range(2):
                    yps = ps_y.tile([128, LCMAX], f32, tag="y",
                                    name=f"y{hh}_{i}")
                    for mi, ks in enumerate((kr_t, ki_t)):
                        for kp in range(2):
                            blkidx = (mi * 2 + kp) * 2 + hh
                            nc.tensor.matmul(
                                yps[:, 0:lc],
                                wc_sb[:, blkidx * 128:(blkidx + 1) * 128],
                                ks[kp][:, 0:lc], start=(mi == 0 and kp == 0),
                                stop=False)
                    # D*l1w*x rank-1 term as a K=1 matmul
                    nc.tensor.matmul(
                        yps[:, 0:lc], dl1_sb[0:1, hh * 128:(hh + 1) * 128],
                        xb[0:1, 0:lc], start=False, stop=True)
                    if has_yc:
                        nc.vector.tensor_add(
                            yps[:, 0:lc], yps[:, 0:lc],
                            yc_t[:, hh * lc:(hh + 1) * lc])
                    th = wpool.tile([128, LCMAX], f32r, tag=f"th{hh}",
                                    name=f"th{hh}_{i}")
                    nc.scalar.activation(th[:, 0:lc],
                                         yps[:, 0:lc], AF.Tanh)
                    h1 = wpool.tile([128, LCMAX], bf16, tag=f"h1{hh}",
                                    name=f"h1{hh}_{i}")
                    nc.vector.scalar_tensor_tensor(
                        h1[:, 0:lc], xb[:, 0:lc],
                        l1w_sb[:, hh:hh + 1], th[:, 0:lc].bitcast(f32),
                        OP.mult, OP.add)
                    if has_l1b:
                        nc.vector.tensor_scalar_add(
                            h1[:, 0:lc], h1[:, 0:lc],
                            l1b_sb[:, hh:hh + 1])
                    if h1f is not None:
                        if has_l1b:
                            nc.vector.tensor_copy(
                                h1f[:, hh * lc:hh * lc + lc], h1[:, 0:lc])
                        else:
                            nc.vector.scalar_tensor_tensor(
                                h1f[:, hh * lc:hh * lc + lc], xb[:, 0:lc],
                                l1w_sb[:, hh:hh + 1],
                                th[:, 0:lc].bitcast(f32),
                                OP.mult, OP.add)
                    th_t.append(th)
                    h1_t.append(h1)
                return th_t, h1_t, h1f, xb

            # ------------- prologue -------------
            streams = [None] * NCH
            pa = [None] * NCH
            streams[0] = stream_chunk(0)
            streams[1] = stream_chunk(1)
            warm(12)

            # large resident weights
            encbf_sb = cpool.tile([128, NBF * 256], bf16)
            nc.gpsimd.dma_start(out=encbf_sb[:], in_=encbf[:])
            encf8_sb = cpool.tile([128, NF8 * 256], f8)
            nc.gpsimd.dma_start(out=encf8_sb[:], in_=encf8[:])

            warm(6)
            pa[0] = part_a(0, streams[0])
            warm(10)
            pa[1] = part_a(1, streams[1])
            warm(10)

            cp = [None] * NCH
            cp[0] = cproj(0, pa[0])

            pending_tail = None

            def mlp(i, orow, mid=None):
                lo, lc = los[i], CHUNKS[i]
                th_t, h1_t, h1f, xb = cp[i]
                for hh in range(2):
                    nc.tensor.matmul(orow[0:1, 0:lc],
                                     l2v_sb[:, hh:hh + 1],
                                     th_t[hh][:, 0:lc],
                                     start=(hh == 0), stop=False)
                # (l2.l1w)*x rank-1 term as a K=1 matmul
                nc.tensor.matmul(orow[0:1, 0:lc], kapb_sb[0:1, 0:1],
                                 xb[0:1, 0:lc], start=False, stop=False)
                h1f2 = None if h1f is None else \
                    h1f[:, 0:2 * lc].rearrange("p (two l) -> p two l", two=2)
                f_prev = None
                for g in range(NG):
                    eps = ps_e.tile([128, 2 * LCMAX], f32, tag="e",
                                    name=f"eps{g}_{i}")
                    for half in range(2):
                        cc = 2 * g + half
                        if cc >= NBF:
                            j = cc - NBF
                            nc.tensor.matmul(
                                eps[:, half * lc:half * lc + lc],
                                encf8_sb[:, j * 256:(j + 1) * 256].rearrange(
                                    "p (two m) -> p two m", two=2),
                                h1f2, start=True, stop=True,
                                perf_mode=PM.DoubleRow)
                            continue
                        for kh in range(2):
                            nc.tensor.matmul(
                                eps[:, half * lc:half * lc + lc],
                                encbf_sb[:, cc * 256 + kh * 128:
                                         cc * 256 + (kh + 1) * 128],
                                h1_t[kh][:, 0:lc],
                                start=(kh == 0), stop=(kh == 1))
                    f_t = fpool.tile([128, 2 * LCMAX], f32r, tag="f",
                                     name=f"f{g}_{i}")
                    if has_encb:
                        for half in range(2):
                            cc = 2 * g + half
                            nc.scalar.activation(
                                f_t[:, half * lc:half * lc + lc],
                                eps[:, half * lc:half * lc + lc], AF.Tanh,
                                bias=encb_sb[:, cc:cc + 1])
                    else:
                        nc.scalar.activation(f_t[:, 0:2 * lc],
                                             eps[:, 0:2 * lc], AF.Tanh)
                    if f_prev is not None:
                        gp = g - 1
                        for half in range(2):
                            cc = 2 * gp + half
                            nc.tensor.matmul(
                                orow[0:1, 0:lc], v_sb[:, cc:cc + 1],
                                f_prev[:, half * lc:half * lc + lc],
                                start=False, stop=False)
                    f_prev = f_t
                    if g == 2 and mid is not None:
                        mid()
                return f_prev

            for i in range(NCH):
                lo, lc = los[i], CHUNKS[i]
                xb = cp[i][3]

                # streams two chunks ahead; d1 muls go on the DVE queue
                # before h1 so the scan pipeline isn't head-of-line blocked
                d1_next = None
                if i + 2 < NCH:
                    streams[i + 2] = stream_chunk(i + 2)
                    d1_next = part_a_d1(i + 2, streams[i + 2])

                orow = ps_o.tile([1, LCMAX], f32, tag="or", name=f"or_{i}")

                # During the fill the DVE scan pipeline lags the PE, so
                # the next C-projection would head-of-line-block the PE;
                # emit it after this chunk's MLP instead.  From chunk 3
                # on (DVE caught up) emit it first (steady state).  The
                # deferred v-dot tail goes after the next C-projection so
                # the PE never waits on the last tanh group.
                early = i < 3
                if early:
                    if pending_tail is not None:
                        pending_tail()
                        pending_tail = None
                    f_prev = mlp(i, orow)
                    if i + 1 < NCH:
                        cp[i + 1] = cproj(i + 1, pa[i + 1])
                else:
                    if i + 1 < NCH:
                        cp[i + 1] = cproj(i + 1, pa[i + 1])
                    if pending_tail is not None:
                        pending_tail()
                        pending_tail = None
                    f_prev = mlp(i, orow)

                if i + 2 < NCH:
                    pa[i + 2] = part_a(i + 2, streams[i + 2], d1=d1_next)

                def tail(i=i, lo=lo, lc=lc, orow=orow, f_prev=f_prev):
                    for half in range(2):
                        cc = 2 * (NG - 1) + half
                        nc.tensor.matmul(
                            orow[0:1, 0:lc], v_sb[:, cc:cc + 1],
                            f_prev[:, half * lc:half * lc + lc],
                            start=False, stop=(half == 1))
                    og = wpool.tile([1, LCMAX], f32, tag="og",
                                    name=f"og_{i}")
                    if has_c0:
                        nc.scalar.activation(og[0:1, 0:lc], orow[0:1, 0:lc],
                                             AF.Identity,
                                             bias=kap_sb[0:1, 1:2])
                    else:
                        nc.scalar.activation(og[0:1, 0:lc], orow[0:1, 0:lc],
                                             AF.Identity)
                    nc.sync.dma_start(out=out[0:1, lo:lo + lc],
                                      in_=og[0:1, 0:lc])
                pending_tail = tail

            pending_tail()

    _split_multi_waits(nc)
    return nc


def kernel(x, l1_w, l1_b, lam_re, lam_im, B_re, B_im, C_re, C_im, D,
           log_step, ff_enc_w, ff_enc_b, ff_dec_w, ff_dec_b, l2_w, l2_b):
    bf = ml_dtypes.bfloat16
    e4 = ml_dtypes.float8_e4m3fn
    x = np.asarray(x, np.float32)
    B = x.shape[0]
    t = derive_host_tables(l1_w, l1_b, lam_re, lam_im, B_re, B_im,
                           C_re, C_im, D, log_step)

    l2row = np.asarray(l2_w, np.float64)[0]            # [H]
    dec_w = np.asarray(ff_dec_w, np.float64)           # [H, CE]
    v = (l2row @ dec_w).astype(np.float32)             # [CE]

    # permute MLP channels by |v|: the NF8*128 smallest go to the fp8
    # cc blocks (placed last), so fp8 quantization noise lands on the
    # channels with the least output weight.
    perm = np.argsort(np.abs(v))                       # ascending |v|
    perm = np.concatenate([perm[NF8 * 128:], perm[:NF8 * 128]])
    v_p = v[perm]
    vpk = v_p.reshape(NCC, 128).T.copy()               # [128, NCC]

    enc_w = np.asarray(ff_enc_w, np.float32)[perm]     # [CE, H] permuted
    ET = enc_w.T                                       # [H, CE]
    encbf = np.empty((128, NBF * 256), np.float32)
    for cc in range(NBF):
        for kh in range(2):
            encbf[:, cc * 256 + kh * 128:cc * 256 + (kh + 1) * 128] = \
                ET[kh * 128:(kh + 1) * 128, cc * 128:(cc + 1) * 128]
    encf8 = np.empty((128, NF8 * 256), np.float32)
    for j in range(NF8):
        cc = NBF + j
        for kh in range(2):
            encf8[:, j * 256 + kh * 128:j * 256 + (kh + 1) * 128] = \
                ET[kh * 128:(kh + 1) * 128, cc * 128:(cc + 1) * 128]
    encbf = encbf.astype(bf).copy()
    encf8 = encf8.astype(e4).copy()
    encb_v = np.asarray(ff_enc_b, np.float32)[perm]
    l2vc = np.asarray(l2_w, np.float32)[0].reshape(2, 128).T.copy()
    l1wc = t['l1w'].reshape(2, 128).T.copy()
    kappa = float(l2row @ np.asarray(l1_w, np.float64)[:, 0])
    c0 = float(l2row @ np.asarray(ff_dec_b, np.float64)
               + np.asarray(l2_b, np.float64)[0]
               + l2row @ np.asarray(l1_b, np.float64))
    kap = np.array([[kappa, c0]], np.float32)
    kapb = np.array([[kappa]], np.float32).astype(bf)

    flags = dict(
        has_yc=t['yc'] is not None,
        has_encb=bool(np.any(encb_v != 0)),
        has_c0=(c0 != 0.0),
        has_l1b=bool(np.any(t['l1b'] != 0)),
        tab_offs=t['tab_offs'],
    )
    nc = build_program(flags)

    shared = dict(tabs=t['tabs'], rbc=t['rbc'], wc=t['wc'],
                  dl1row=t['dl1row'], encbf=encbf, encf8=encf8,
                  vpk=vpk, l2v=l2vc, l1wc=l1wc, kap=kap, kapb=kapb)
    if flags['has_encb']:
        shared['encb'] = encb_v.reshape(NCC, 128).T.copy()
    if flags['has_l1b']:
        shared['l1bc'] = t['l1b'].reshape(2, 128).T.copy()
    if flags['has_yc']:
        shared['yc'] = t['yc']
    in_maps = []
    for b in range(B):
        xb = np.ascontiguousarray(x[b, :, 0]).astype(bf)[None, :]  # [1, L]
        m = dict(shared)
        m['xbs'] = xb
        in_maps.append(m)

    res = run_bass_kernel_spmd(nc, in_maps, list(range(B)))
    outs = [res.results[b]["out"][0][:, None] for b in range(B)]
    return np.stack(outs).astype(np.float32)


if __name__ == "__main__":
    pass

